# revision 28
# baseline (speedup 1.0000x reference)
"""DistMult edge scoring on Trainium2 (8 NeuronCores).

score_e = src_emb[e]^T @ W[rel_e] @ dst_emb[e]   for E=100k edges.

Strategy
--------
Host (index-space preprocessing only — no embedding data is gathered on
host):
  - Sort edges by relation, shard the sorted list contiguously across the
    8 cores (data-parallel over edges).
  - Per core, bucket edges into 16 segments by (src_bank, dst_bank) where
    a bank is a num_nodes/4-row range of the node table (node ids must fit
    the int16 index of the HW gather instruction). Within a segment, edges
    stay relation-sorted and each relation run is padded to a multiple of
    128 so every 128-edge tile is single-relation and single-bank on both
    endpoints. Segment capacities are maxed across cores so all cores run
    one SPMD program.
  - The small relation matrices (64x64x64 = 1MB) are expanded per tile on
    host and streamed over the HWDGE path, keeping the SWDGE gather
    queues free for the 25.6MB node table.

Device (per core, SPMD):
  - dma_gather (int16, 4 SWDGE queues, <=1024 rows/instruction) of
    src/dst embedding rows, per bank segment.
  - per tile: PE transpose src -> srcT [d, e] (base partition 0);
    PE matmul U[e,k] = sum_d srcT[d,e] * W[r,d,k];
    per 8 tiles: DVE mul by dst and reduce over k -> scores.
Host: drop pad slots, unsort scores to the original edge order.
"""

import numpy as np

import concourse.bacc as bacc
import concourse.mybir as mybir
from concourse.bass_utils import run_bass_kernel_spmd
from concourse.masks import make_identity
from concourse.tile import TileContext

NCORES = 8
P = 128          # SBUF partitions / edges per tile
DIM = 64         # embedding dim
NUM_RELS = 64
NBANKS = 4
TILE_GROUP = 8   # tiles per compute super-tile (one PSUM bank: 8*64 f32)
CHUNK = 1024     # max indices per dma_gather instruction (SWDGE ring limit)
NQ = 4           # SWDGE queues

TRACE = False
LAST_RESULT = None

_BUILD_CACHE = {}


def _prepare(triplets, num_nodes):
    """Index-space prep. Returns per-core int16 gather streams + unsort map.

    Slot s (= tile j * 128 + p) of core c holds the edge at padded position
    s; tiles are relation- and bank-pure by construction.
    """
    t = np.asarray(triplets)
    E = t.shape[0]
    src = t[:, 0].astype(np.int64)
    rel = t[:, 1].astype(np.int64)
    dst = t[:, 2].astype(np.int64)
    bank = -(-num_nodes // NBANKS)  # equal banks, < 32768 for int16
    assert bank <= 32767

    order = np.argsort(rel, kind="stable")
    bounds = [round(c * E / NCORES) for c in range(NCORES + 1)]

    # segments by src bank only; dst goes through a two-stage gather
    core_data = []
    for c in range(NCORES):
        eidx = order[bounds[c]:bounds[c + 1]]
        seg = src[eidx] // bank
        segs = []
        for s in range(NBANKS):
            sel = eidx[seg == s]           # still rel-sorted (stable mask)
            r = rel[sel]
            n = len(sel)
            if n:
                change = np.flatnonzero(np.diff(r)) + 1
                starts = np.concatenate([[0], change])
                ends = np.concatenate([change, [n]])
                lens = ends - starts
                padlens = ((lens + P - 1) // P) * P
                offs = np.concatenate([[0], np.cumsum(padlens)])
                total = int(offs[-1])
                se = np.full(total, -1, np.int64)
                pos = (np.arange(n) - np.repeat(starts, lens)
                       + np.repeat(offs[:-1], lens))
                se[pos] = sel
                sr = np.repeat(r[starts], padlens)
            else:
                se = np.zeros(0, np.int64)
                sr = np.zeros(0, np.int64)
            segs.append((se, sr))
        core_data.append(segs)

    caps = []
    for s in range(NBANKS):
        cap = max(len(core_data[c][s][0]) for c in range(NCORES)) // P
        caps.append(int(cap))
    K = sum(caps)
    pad_tiles = (-K) % TILE_GROUP  # compute loop works in groups of 8 tiles
    caps[-1] += pad_tiles
    K += pad_tiles
    seg_tile0 = np.concatenate([[0], np.cumsum(caps)]).astype(int)

    nslot = K * P
    src_loc = np.zeros((NCORES, nslot), np.int64)   # bank-local src idx
    dst_all = np.zeros((NCORES, nslot), np.int64)   # global dst ids
    dst_valid = np.zeros((NCORES, nslot), bool)
    relc_all = np.zeros((NCORES, K), np.int64)      # relation per tile
    slot_edge_full = np.full((NCORES, nslot), -1, np.int64)

    for c in range(NCORES):
        for s in range(NBANKS):
            se, sr = core_data[c][s]
            a = seg_tile0[s] * P
            m = len(se)
            slot_edge_full[c, a:a + m] = se
            valid = se >= 0
            sl = np.zeros(m, np.int64)
            sl[valid] = src[se[valid]] - s * bank
            src_loc[c, a:a + m] = sl
            dl = np.zeros(m, np.int64)
            dl[valid] = dst[se[valid]]
            dst_all[c, a:a + m] = dl
            dst_valid[c, a:a + m] = valid
            nt = m // P
            if nt:
                relc_all[c, seg_tile0[s]:seg_tile0[s] + nt] = \
                    sr.reshape(nt, P)[:, 0]

    # dst stage A: per dst bank, the slots needing that bank, compacted.
    # Per-bank capacity maxed over cores; total rows CT (scratch size).
    bank_cnt = np.zeros((NCORES, NBANKS), np.int64)
    for c in range(NCORES):
        for b in range(NBANKS):
            sel = dst_valid[c] & (dst_all[c] // bank == b)
            bank_cnt[c, b] = len(np.unique(dst_all[c, sel]))
    bcaps = [int(-(-bank_cnt[:, b].max() // P) * P) for b in range(NBANKS)]
    brow0 = np.concatenate([[0], np.cumsum(bcaps)]).astype(int)
    CT = int(brow0[-1])  # scratch rows; must fit int16 for stage B
    assert CT <= 32767 and CT % P == 0

    dstA_loc = np.zeros((NCORES, CT), np.int64)   # stage-A bank-local ids
    dstB_row = np.zeros((NCORES, nslot), np.int64)  # stage-B scratch rows
    BC = CT // P  # scratch free blocks per partition
    for c in range(NCORES):
        for b in range(NBANKS):
            sel = np.flatnonzero(dst_valid[c] & (dst_all[c] // bank == b))
            # dedup: gather each distinct row once, point dup slots at it
            uniq, inv = np.unique(dst_all[c, sel], return_inverse=True)
            g = brow0[b] + np.arange(len(uniq))    # stage-A request index
            dstA_loc[c, g] = uniq - b * bank
            # request g lands at scratch flat row (g%128)*BC + g//128
            gslot = g[inv]
            dstB_row[c, sel] = (gslot % P) * BC + gslot // P

    def to_idx_tile(flat):
        n = flat.shape[1]
        blk = flat.reshape(NCORES, n // 16, 16).transpose(0, 2, 1)
        return np.tile(blk, (1, 8, 1)).astype(np.int16)

    src_idx = to_idx_tile(src_loc)
    dstA_idx = to_idx_tile(dstA_loc)
    dstB_idx = to_idx_tile(dstB_row)

    def chunk_plan(ranges):
        plan = []
        for tag, a, b in ranges:
            g0 = a
            while g0 < b:
                n = min(CHUNK, b - g0)
                plan.append((tag, g0, n))
                g0 += n
        return tuple(plan)

    src_plan = chunk_plan(
        [(s, seg_tile0[s] * P, seg_tile0[s + 1] * P) for s in range(NBANKS)])
    dstA_plan = chunk_plan(
        [(b, brow0[b], brow0[b + 1]) for b in range(NBANKS)])
    dstB_plan = chunk_plan([(0, 0, nslot)])

    return (src_idx, dstA_idx, dstB_idx, relc_all, slot_edge_full, K, CT,
            src_plan, dstA_plan, dstB_plan, E)


def _build(K, CT, num_nodes, src_plan, dstA_plan, dstB_plan):
    nc = bacc.Bacc("TRN2", target_bir_lowering=False, debug=False,
                   num_devices=NCORES, num_swdge_queues=NQ)
    f32, i16 = mybir.dt.float32, mybir.dt.int16
    bank = -(-num_nodes // NBANKS)
    nslot = K * P
    BC = CT // P
    HG = TILE_GROUP // 2  # tiles per transpose/copy batch (one PSUM bank)

    node = nc.dram_tensor("node_emb", [num_nodes, DIM], f32,
                          kind="ExternalInput")
    wt_d = nc.dram_tensor("w_tile", [DIM, K * DIM], f32,
                          kind="ExternalInput")
    sidx_d = nc.dram_tensor("src_idx", [P, nslot // 16], i16,
                            kind="ExternalInput")
    daidx_d = nc.dram_tensor("dstA_idx", [P, CT // 16], i16,
                             kind="ExternalInput")
    dbidx_d = nc.dram_tensor("dstB_idx", [P, nslot // 16], i16,
                             kind="ExternalInput")
    out_d = nc.dram_tensor("scores", [P, K], f32, kind="ExternalOutput")

    with TileContext(nc) as tc:
        with (
            tc.tile_pool(name="persist", bufs=1) as persist,
            tc.tile_pool(name="tsb", bufs=4) as tsb_pool,
            tc.tile_pool(name="pbig", bufs=3) as pbig_pool,
            tc.tile_pool(name="dram", bufs=1, space="DRAM") as dram_pool,
            tc.tile_pool(name="tpsum", bufs=2, space="PSUM") as tpsum_pool,
            tc.tile_pool(name="upsum", bufs=4, space="PSUM") as upsum_pool,
        ):
            sidx = persist.tile([P, nslot // 16], i16, tag="sidx")
            daidx = persist.tile([P, CT // 16], i16, tag="daidx")
            dbidx = persist.tile([P, nslot // 16], i16, tag="dbidx")
            ident = persist.tile([P, P], f32, tag="ident")
            src_g = persist.tile([P, K * DIM], f32, tag="src_g")
            dstA = persist.tile([P, BC * DIM], f32, tag="dstA")
            dst_g = persist.tile([P, K * DIM], f32, tag="dst_g")
            w_g = persist.tile([DIM, K * DIM], f32, tag="w_g")
            scores = persist.tile([P, K], f32, tag="scores")
            scratch = dram_pool.tile([P, BC * DIM], f32, tag="scratch")

            nc.sync.dma_start(out=sidx[:], in_=sidx_d[:])
            nc.sync.dma_start(out=daidx[:], in_=daidx_d[:])
            nc.sync.dma_start(out=dbidx[:], in_=dbidx_d[:])
            nc.sync.dma_start(out=w_g[:], in_=wt_d[:])
            make_identity(nc, ident[:])

            # Interleave src and dst-stage-A chunks so the dst pipeline
            # drains early and stage B can start while src still gathers.
            # queue_num must follow the global SWDGE round-robin (Tile locks
            # DMA sem lanes to queues by instruction order).
            qn = 0
            src_items = [("s",) + it for it in src_plan]
            dstA_items = [("a",) + it for it in dstA_plan]
            inter = []
            na, nb = len(dstA_items), len(src_items)
            ia = ib = 0
            while ia < na or ib < nb:
                if ia < na:
                    inter.append(dstA_items[ia]); ia += 1
                if ib < nb:
                    inter.append(src_items[ib]); ib += 1
            for kind, bnk, g0, n in inter:
                hi = min(num_nodes, (bnk + 1) * bank)
                g_tile, idx_tile = ((src_g, sidx) if kind == "s"
                                    else (dstA, daidx))
                nc.gpsimd.dma_gather(
                    g_tile[:, (g0 // P) * DIM:((g0 + n) // P) * DIM]
                    .rearrange("p (t d) -> p t d", d=DIM),
                    node[bnk * bank:hi, :],
                    idx_tile[:, g0 // 16:(g0 + n) // 16],
                    n, n, DIM,
                    queue_num=qn % NQ,
                )
                qn += 1
            # stage A -> DRAM scratch (sequential, HWDGE path)
            nc.sync.dma_start(out=scratch[:], in_=dstA[:])
            # dst stage B: regather scratch rows into slot order
            scratch_rows = scratch[:].rearrange("a (b c) -> (a b) c", c=DIM)
            for _, g0, n in dstB_plan:
                nc.gpsimd.dma_gather(
                    dst_g[:, (g0 // P) * DIM:((g0 + n) // P) * DIM]
                    .rearrange("p (t d) -> p t d", d=DIM),
                    scratch_rows,
                    dbidx[:, g0 // 16:(g0 + n) // 16],
                    n, n, DIM,
                    queue_num=qn % NQ,
                )
                qn += 1

            for st in range(K // TILE_GROUP):
                t0 = st * TILE_GROUP
                # 4 pair-transposes ([128,128] -> [dA|dB, e]) into one PSUM
                # bank, then two strided ACT copies deinterleave the halves
                # into a base-partition-0 srcT buffer [64, 8*128].
                tp = tpsum_pool.tile([P, HG * P], f32, tag="tp")
                for q in range(HG):
                    c0 = (t0 + 2 * q) * DIM
                    nc.tensor.transpose(
                        out=tp[:, q * P:(q + 1) * P],
                        in_=src_g[:, c0:c0 + 2 * DIM],
                        identity=ident[:],
                    )
                tsb = tsb_pool.tile([DIM, TILE_GROUP * P], f32, tag="tsb")
                tsb_v = tsb[:].rearrange("p (t a c) -> p a t c", a=2, c=P)
                tp_v = tp[:].rearrange("p (q c) -> p q c", c=P)
                nc.scalar.copy(out=tsb_v[:, 0], in_=tp_v[0:DIM])
                nc.scalar.copy(out=tsb_v[:, 1], in_=tp_v[DIM:P])

                u = upsum_pool.tile([P, TILE_GROUP * DIM], f32, tag="u")
                for h in range(TILE_GROUP):
                    j = t0 + h
                    nc.tensor.matmul(
                        out=u[:, h * DIM:(h + 1) * DIM],
                        lhsT=tsb[:, h * P:(h + 1) * P],
                        rhs=w_g[:, j * DIM:(j + 1) * DIM],
                        start=True,
                        stop=True,
                    )
                pbig = pbig_pool.tile([P, TILE_GROUP * DIM], f32, tag="pbig")
                nc.vector.tensor_mul(
                    out=pbig[:],
                    in0=u[:],
                    in1=dst_g[:, t0 * DIM:(t0 + TILE_GROUP) * DIM],
                )
                nc.vector.reduce_sum(
                    out=scores[:, t0:t0 + TILE_GROUP],
                    in_=pbig[:].rearrange("p (t k) -> p t k", k=DIM),
                    axis=mybir.AxisListType.X,
                )

            nc.sync.dma_start(out=out_d[:], in_=scores[:])

    nc.compile()
    return nc


def kernel(triplets, node_emb, W):
    global LAST_RESULT
    node = np.ascontiguousarray(np.asarray(node_emb, dtype=np.float32))
    Wf = np.ascontiguousarray(np.asarray(W, dtype=np.float32))
    num_nodes = node.shape[0]

    (src_idx, dstA_idx, dstB_idx, relc_all, slot_edge, K, CT,
     src_plan, dstA_plan, dstB_plan, E) = _prepare(triplets, num_nodes)

    cache_key = (K, CT, num_nodes, src_plan, dstA_plan, dstB_plan)
    if cache_key not in _BUILD_CACHE:
        _BUILD_CACHE[cache_key] = _build(K, CT, num_nodes, src_plan,
                                         dstA_plan, dstB_plan)
    nc = _BUILD_CACHE[cache_key]

    in_maps = []
    for c in range(NCORES):
        # per-tile W: [K, 64, 64] -> [64, K*64] with w[d, j*64+k] = W[rel_j,d,k]
        wt = np.ascontiguousarray(
            Wf[relc_all[c]].transpose(1, 0, 2).reshape(DIM, K * DIM))
        in_maps.append({
            "node_emb": node,
            "w_tile": wt,
            "src_idx": np.ascontiguousarray(src_idx[c]),
            "dstA_idx": np.ascontiguousarray(dstA_idx[c]),
            "dstB_idx": np.ascontiguousarray(dstB_idx[c]),
        })

    res = run_bass_kernel_spmd(nc, in_maps, list(range(NCORES)), trace=TRACE)
    LAST_RESULT = res

    out = np.zeros(E, np.float32)
    for c in range(NCORES):
        sc = np.asarray(res.results[c]["scores"])  # [P, K]
        flat = sc.T.ravel()                        # index j*P+p = slot s
        se = slot_edge[c]
        valid = se >= 0
        out[se[valid]] = flat[valid]
    return out


# revision 29
# speedup vs baseline: 1.3115x; 1.3115x over previous
"""DistMult edge scoring on Trainium2 (8 NeuronCores).

score_e = src_emb[e]^T @ W[rel_e] @ dst_emb[e]   for E=100k edges.

Strategy
--------
Host (index-space preprocessing only — no embedding data is gathered on
host):
  - Sort edges by relation, shard the sorted list contiguously across the
    8 cores (data-parallel over edges).
  - Per core, bucket edges into 16 segments by (src_bank, dst_bank) where
    a bank is a num_nodes/4-row range of the node table (node ids must fit
    the int16 index of the HW gather instruction). Within a segment, edges
    stay relation-sorted and each relation run is padded to a multiple of
    128 so every 128-edge tile is single-relation and single-bank on both
    endpoints. Segment capacities are maxed across cores so all cores run
    one SPMD program.
  - The small relation matrices (64x64x64 = 1MB) are expanded per tile on
    host and streamed over the HWDGE path, keeping the SWDGE gather
    queues free for the 25.6MB node table.

Device (per core, SPMD):
  - dma_gather (int16, 4 SWDGE queues, <=1024 rows/instruction) of
    src/dst embedding rows, per bank segment.
  - per tile: PE transpose src -> srcT [d, e] (base partition 0);
    PE matmul U[e,k] = sum_d srcT[d,e] * W[r,d,k];
    per 8 tiles: DVE mul by dst and reduce over k -> scores.
Host: drop pad slots, unsort scores to the original edge order.
"""

import numpy as np

import concourse.bacc as bacc
import concourse.mybir as mybir
from concourse.bass_utils import run_bass_kernel_spmd
from concourse.masks import make_identity
from concourse.tile import TileContext

NCORES = 8
P = 128          # SBUF partitions / edges per tile
DIM = 64         # embedding dim
NUM_RELS = 64
NBANKS = 4
TILE_GROUP = 8   # tiles per compute super-tile (one PSUM bank: 8*64 f32)
CHUNK = 1024     # max indices per dma_gather instruction (SWDGE ring limit)
NQ = 4           # SWDGE queues

TRACE = False
LAST_RESULT = None

_BUILD_CACHE = {}


def _prepare(triplets, num_nodes):
    """Index-space prep. Returns per-core int16 gather streams + unsort map.

    Slot s (= tile j * 128 + p) of core c holds the edge at padded position
    s; tiles are relation- and bank-pure by construction.
    """
    t = np.asarray(triplets)
    E = t.shape[0]
    src = t[:, 0].astype(np.int64)
    rel = t[:, 1].astype(np.int64)
    dst = t[:, 2].astype(np.int64)
    bank = -(-num_nodes // NBANKS)  # equal banks, < 32768 for int16
    assert bank <= 32767

    order = np.argsort(rel, kind="stable")
    bounds = [round(c * E / NCORES) for c in range(NCORES + 1)]

    # segments by src bank only; dst goes through a two-stage gather
    core_data = []
    for c in range(NCORES):
        eidx = order[bounds[c]:bounds[c + 1]]
        seg = src[eidx] // bank
        segs = []
        for s in range(NBANKS):
            sel = eidx[seg == s]           # still rel-sorted (stable mask)
            r = rel[sel]
            n = len(sel)
            if n:
                change = np.flatnonzero(np.diff(r)) + 1
                starts = np.concatenate([[0], change])
                ends = np.concatenate([change, [n]])
                lens = ends - starts
                padlens = ((lens + P - 1) // P) * P
                offs = np.concatenate([[0], np.cumsum(padlens)])
                total = int(offs[-1])
                se = np.full(total, -1, np.int64)
                pos = (np.arange(n) - np.repeat(starts, lens)
                       + np.repeat(offs[:-1], lens))
                se[pos] = sel
                sr = np.repeat(r[starts], padlens)
            else:
                se = np.zeros(0, np.int64)
                sr = np.zeros(0, np.int64)
            segs.append((se, sr))
        core_data.append(segs)

    caps = []
    for s in range(NBANKS):
        cap = max(len(core_data[c][s][0]) for c in range(NCORES)) // P
        caps.append(int(cap))
    K = sum(caps)
    pad_tiles = (-K) % TILE_GROUP  # compute loop works in groups of 8 tiles
    caps[-1] += pad_tiles
    K += pad_tiles
    seg_tile0 = np.concatenate([[0], np.cumsum(caps)]).astype(int)

    nslot = K * P
    src_loc = np.zeros((NCORES, nslot), np.int64)   # bank-local src idx
    dst_all = np.zeros((NCORES, nslot), np.int64)   # global dst ids
    dst_valid = np.zeros((NCORES, nslot), bool)
    relc_all = np.zeros((NCORES, K), np.int64)      # relation per tile
    slot_edge_full = np.full((NCORES, nslot), -1, np.int64)

    for c in range(NCORES):
        for s in range(NBANKS):
            se, sr = core_data[c][s]
            a = seg_tile0[s] * P
            m = len(se)
            slot_edge_full[c, a:a + m] = se
            valid = se >= 0
            sl = np.zeros(m, np.int64)
            sl[valid] = src[se[valid]] - s * bank
            src_loc[c, a:a + m] = sl
            dl = np.zeros(m, np.int64)
            dl[valid] = dst[se[valid]]
            dst_all[c, a:a + m] = dl
            dst_valid[c, a:a + m] = valid
            nt = m // P
            if nt:
                relc_all[c, seg_tile0[s]:seg_tile0[s] + nt] = \
                    sr.reshape(nt, P)[:, 0]

    # dst stage A: per dst bank, the slots needing that bank, compacted.
    # Per-bank capacity maxed over cores; total rows CT (scratch size).
    bank_cnt = np.zeros((NCORES, NBANKS), np.int64)
    for c in range(NCORES):
        for b in range(NBANKS):
            sel = dst_valid[c] & (dst_all[c] // bank == b)
            bank_cnt[c, b] = len(np.unique(dst_all[c, sel]))
    bcaps = [int(-(-bank_cnt[:, b].max() // P) * P) for b in range(NBANKS)]
    brow0 = np.concatenate([[0], np.cumsum(bcaps)]).astype(int)
    CT = int(brow0[-1])  # scratch rows; must fit int16 for stage B
    assert CT <= 32767 and CT % P == 0

    dstA_loc = np.zeros((NCORES, CT), np.int64)   # stage-A bank-local ids
    dstB_row = np.zeros((NCORES, nslot), np.int64)  # stage-B scratch rows
    BC = CT // P  # scratch free blocks per partition
    for c in range(NCORES):
        for b in range(NBANKS):
            sel = np.flatnonzero(dst_valid[c] & (dst_all[c] // bank == b))
            # dedup: gather each distinct row once, point dup slots at it
            uniq, inv = np.unique(dst_all[c, sel], return_inverse=True)
            g = brow0[b] + np.arange(len(uniq))    # stage-A request index
            dstA_loc[c, g] = uniq - b * bank
            # request g lands at scratch flat row (g%128)*BC + g//128
            gslot = g[inv]
            dstB_row[c, sel] = (gslot % P) * BC + gslot // P

    def to_idx_tile(flat):
        n = flat.shape[1]
        blk = flat.reshape(NCORES, n // 16, 16).transpose(0, 2, 1)
        return np.tile(blk, (1, 8, 1)).astype(np.int16)

    src_idx = to_idx_tile(src_loc)
    dstA_idx = to_idx_tile(dstA_loc)
    dstB_idx = to_idx_tile(dstB_row)

    def chunk_plan(ranges, head=()):
        """head: sizes for the leading chunks of the first range (smaller
        chunks let the consumer start sooner)."""
        plan = []
        first = True
        for tag, a, b in ranges:
            g0 = a
            if first:
                for h in head:
                    if g0 + h > b:
                        break
                    plan.append((tag, g0, h))
                    g0 += h
                first = False
            while g0 < b:
                n = min(CHUNK, b - g0)
                plan.append((tag, g0, n))
                g0 += n
        return tuple(plan)

    # small leading src chunks: the PE pipeline (transpose->matmul) can
    # start as soon as the first 256 rows land instead of waiting ~9us
    src_plan = chunk_plan(
        [(s, seg_tile0[s] * P, seg_tile0[s + 1] * P) for s in range(NBANKS)],
        head=(256, 256, 512))
    dstA_plan = chunk_plan(
        [(b, brow0[b], brow0[b + 1]) for b in range(NBANKS)])
    # finer stage-B chunks: the per-supertile DVE stage consumes dst in
    # 1024-slot groups, so 512-row chunks smooth the tail
    dstB_plan = tuple(
        (0, g0, min(512, nslot - g0)) for g0 in range(0, nslot, 512))

    return (src_idx, dstA_idx, dstB_idx, relc_all, slot_edge_full, K, CT,
            src_plan, dstA_plan, dstB_plan, E)


def _build(K, CT, num_nodes, src_plan, dstA_plan, dstB_plan):
    nc = bacc.Bacc("TRN2", target_bir_lowering=False, debug=False,
                   num_devices=NCORES, num_swdge_queues=NQ)
    f32, i16 = mybir.dt.float32, mybir.dt.int16
    bank = -(-num_nodes // NBANKS)
    nslot = K * P
    BC = CT // P
    HG = TILE_GROUP // 2  # tiles per transpose/copy batch (one PSUM bank)

    node = nc.dram_tensor("node_emb", [num_nodes, DIM], f32,
                          kind="ExternalInput")
    wt_d = nc.dram_tensor("w_tile", [DIM, K * DIM], f32,
                          kind="ExternalInput")
    sidx_d = nc.dram_tensor("src_idx", [P, nslot // 16], i16,
                            kind="ExternalInput")
    daidx_d = nc.dram_tensor("dstA_idx", [P, CT // 16], i16,
                             kind="ExternalInput")
    dbidx_d = nc.dram_tensor("dstB_idx", [P, nslot // 16], i16,
                             kind="ExternalInput")
    out_d = nc.dram_tensor("scores", [P, K], f32, kind="ExternalOutput")

    with TileContext(nc) as tc:
        with (
            tc.tile_pool(name="persist", bufs=1) as persist,
            tc.tile_pool(name="tsb", bufs=4) as tsb_pool,
            tc.tile_pool(name="pbig", bufs=3) as pbig_pool,
            tc.tile_pool(name="dram", bufs=1, space="DRAM") as dram_pool,
            tc.tile_pool(name="tpsum", bufs=2, space="PSUM") as tpsum_pool,
            tc.tile_pool(name="upsum", bufs=4, space="PSUM") as upsum_pool,
        ):
            sidx = persist.tile([P, nslot // 16], i16, tag="sidx")
            daidx = persist.tile([P, CT // 16], i16, tag="daidx")
            dbidx = persist.tile([P, nslot // 16], i16, tag="dbidx")
            ident = persist.tile([P, P], f32, tag="ident")
            src_g = persist.tile([P, K * DIM], f32, tag="src_g")
            dstA = persist.tile([P, BC * DIM], f32, tag="dstA")
            dst_g = persist.tile([P, K * DIM], f32, tag="dst_g")
            w_g = persist.tile([DIM, K * DIM], f32, tag="w_g")
            scores = persist.tile([P, K], f32, tag="scores")
            scratch = dram_pool.tile([P, BC * DIM], f32, tag="scratch")

            nc.sync.dma_start(out=sidx[:], in_=sidx_d[:])
            nc.sync.dma_start(out=daidx[:], in_=daidx_d[:])
            nc.sync.dma_start(out=dbidx[:], in_=dbidx_d[:])
            nc.sync.dma_start(out=w_g[:], in_=wt_d[:])
            make_identity(nc, ident[:])

            # Interleave src and dst-stage-A chunks so the dst pipeline
            # drains early and stage B can start while src still gathers.
            # queue_num must follow the global SWDGE round-robin (Tile locks
            # DMA sem lanes to queues by instruction order).
            qn = 0
            src_items = [("s",) + it for it in src_plan]
            dstA_items = [("a",) + it for it in dstA_plan]
            inter = []
            na, nb = len(dstA_items), len(src_items)
            ia = ib = 0
            while ia < na or ib < nb:
                if ia < na:
                    inter.append(dstA_items[ia]); ia += 1
                if ib < nb:
                    inter.append(src_items[ib]); ib += 1
            for kind, bnk, g0, n in inter:
                hi = min(num_nodes, (bnk + 1) * bank)
                g_tile, idx_tile = ((src_g, sidx) if kind == "s"
                                    else (dstA, daidx))
                nc.gpsimd.dma_gather(
                    g_tile[:, (g0 // P) * DIM:((g0 + n) // P) * DIM]
                    .rearrange("p (t d) -> p t d", d=DIM),
                    node[bnk * bank:hi, :],
                    idx_tile[:, g0 // 16:(g0 + n) // 16],
                    n, n, DIM,
                    queue_num=qn % NQ,
                )
                qn += 1
            # stage A -> DRAM scratch (sequential, HWDGE path)
            nc.sync.dma_start(out=scratch[:], in_=dstA[:])
            # dst stage B: regather scratch rows into slot order
            scratch_rows = scratch[:].rearrange("a (b c) -> (a b) c", c=DIM)
            for _, g0, n in dstB_plan:
                nc.gpsimd.dma_gather(
                    dst_g[:, (g0 // P) * DIM:((g0 + n) // P) * DIM]
                    .rearrange("p (t d) -> p t d", d=DIM),
                    scratch_rows,
                    dbidx[:, g0 // 16:(g0 + n) // 16],
                    n, n, DIM,
                    queue_num=qn % NQ,
                )
                qn += 1

            for st in range(K // TILE_GROUP):
                t0 = st * TILE_GROUP
                # 4 pair-transposes ([128,128] -> [dA|dB, e]) into one PSUM
                # bank, then two strided ACT copies deinterleave the halves
                # into a base-partition-0 srcT buffer [64, 8*128].
                tp = tpsum_pool.tile([P, HG * P], f32, tag="tp")
                for q in range(HG):
                    c0 = (t0 + 2 * q) * DIM
                    nc.tensor.transpose(
                        out=tp[:, q * P:(q + 1) * P],
                        in_=src_g[:, c0:c0 + 2 * DIM],
                        identity=ident[:],
                    )
                tsb = tsb_pool.tile([DIM, TILE_GROUP * P], f32, tag="tsb")
                tsb_v = tsb[:].rearrange("p (t a c) -> p a t c", a=2, c=P)
                tp_v = tp[:].rearrange("p (q c) -> p q c", c=P)
                nc.scalar.copy(out=tsb_v[:, 0], in_=tp_v[0:DIM])
                nc.scalar.copy(out=tsb_v[:, 1], in_=tp_v[DIM:P])

                u = upsum_pool.tile([P, TILE_GROUP * DIM], f32, tag="u")
                for h in range(TILE_GROUP):
                    j = t0 + h
                    nc.tensor.matmul(
                        out=u[:, h * DIM:(h + 1) * DIM],
                        lhsT=tsb[:, h * P:(h + 1) * P],
                        rhs=w_g[:, j * DIM:(j + 1) * DIM],
                        start=True,
                        stop=True,
                    )
                pbig = pbig_pool.tile([P, TILE_GROUP * DIM], f32, tag="pbig")
                nc.vector.tensor_mul(
                    out=pbig[:],
                    in0=u[:],
                    in1=dst_g[:, t0 * DIM:(t0 + TILE_GROUP) * DIM],
                )
                nc.vector.reduce_sum(
                    out=scores[:, t0:t0 + TILE_GROUP],
                    in_=pbig[:].rearrange("p (t k) -> p t k", k=DIM),
                    axis=mybir.AxisListType.X,
                )

            nc.sync.dma_start(out=out_d[:], in_=scores[:])

    nc.compile()
    return nc


def kernel(triplets, node_emb, W):
    global LAST_RESULT
    node = np.ascontiguousarray(np.asarray(node_emb, dtype=np.float32))
    Wf = np.ascontiguousarray(np.asarray(W, dtype=np.float32))
    num_nodes = node.shape[0]

    (src_idx, dstA_idx, dstB_idx, relc_all, slot_edge, K, CT,
     src_plan, dstA_plan, dstB_plan, E) = _prepare(triplets, num_nodes)

    cache_key = (K, CT, num_nodes, src_plan, dstA_plan, dstB_plan)
    if cache_key not in _BUILD_CACHE:
        _BUILD_CACHE[cache_key] = _build(K, CT, num_nodes, src_plan,
                                         dstA_plan, dstB_plan)
    nc = _BUILD_CACHE[cache_key]

    in_maps = []
    for c in range(NCORES):
        # per-tile W: [K, 64, 64] -> [64, K*64] with w[d, j*64+k] = W[rel_j,d,k]
        wt = np.ascontiguousarray(
            Wf[relc_all[c]].transpose(1, 0, 2).reshape(DIM, K * DIM))
        in_maps.append({
            "node_emb": node,
            "w_tile": wt,
            "src_idx": np.ascontiguousarray(src_idx[c]),
            "dstA_idx": np.ascontiguousarray(dstA_idx[c]),
            "dstB_idx": np.ascontiguousarray(dstB_idx[c]),
        })

    res = run_bass_kernel_spmd(nc, in_maps, list(range(NCORES)), trace=TRACE)
    LAST_RESULT = res

    out = np.zeros(E, np.float32)
    for c in range(NCORES):
        sc = np.asarray(res.results[c]["scores"])  # [P, K]
        flat = sc.T.ravel()                        # index j*P+p = slot s
        se = slot_edge[c]
        valid = se >= 0
        out[se[valid]] = flat[valid]
    return out


# revision 30
# speedup vs baseline: 1.3157x; 1.0032x over previous
"""DistMult edge scoring on Trainium2 (8 NeuronCores).

score_e = src_emb[e]^T @ W[rel_e] @ dst_emb[e]   for E=100k edges.

Strategy
--------
Host (index-space preprocessing only — no embedding data is gathered on
host):
  - Sort edges by relation, shard the sorted list contiguously across the
    8 cores (data-parallel over edges).
  - Per core, bucket edges into 16 segments by (src_bank, dst_bank) where
    a bank is a num_nodes/4-row range of the node table (node ids must fit
    the int16 index of the HW gather instruction). Within a segment, edges
    stay relation-sorted and each relation run is padded to a multiple of
    128 so every 128-edge tile is single-relation and single-bank on both
    endpoints. Segment capacities are maxed across cores so all cores run
    one SPMD program.
  - The small relation matrices (64x64x64 = 1MB) are expanded per tile on
    host and streamed over the HWDGE path, keeping the SWDGE gather
    queues free for the 25.6MB node table.

Device (per core, SPMD):
  - dma_gather (int16, 4 SWDGE queues, <=1024 rows/instruction) of
    src/dst embedding rows, per bank segment.
  - per tile: PE transpose src -> srcT [d, e] (base partition 0);
    PE matmul U[e,k] = sum_d srcT[d,e] * W[r,d,k];
    per 8 tiles: DVE mul by dst and reduce over k -> scores.
Host: drop pad slots, unsort scores to the original edge order.
"""

import numpy as np

import concourse.bacc as bacc
import concourse.mybir as mybir
from concourse.bass_utils import run_bass_kernel_spmd
from concourse.masks import make_identity
from concourse.tile import TileContext

NCORES = 8
P = 128          # SBUF partitions / edges per tile
DIM = 64         # embedding dim
NUM_RELS = 64
NBANKS = 4
TILE_GROUP = 8   # tiles per compute super-tile (one PSUM bank: 8*64 f32)
CHUNK = 1024     # max indices per dma_gather instruction (SWDGE ring limit)
NQ = 4           # SWDGE queues

TRACE = False
LAST_RESULT = None

_BUILD_CACHE = {}


def _prepare(triplets, num_nodes):
    """Index-space prep. Returns per-core int16 gather streams + unsort map.

    Slot s (= tile j * 128 + p) of core c holds the edge at padded position
    s; tiles are relation- and bank-pure by construction.
    """
    t = np.asarray(triplets)
    E = t.shape[0]
    src = t[:, 0].astype(np.int64)
    rel = t[:, 1].astype(np.int64)
    dst = t[:, 2].astype(np.int64)
    bank = -(-num_nodes // NBANKS)  # equal banks, < 32768 for int16
    assert bank <= 32767

    order = np.argsort(rel, kind="stable")
    bounds = [round(c * E / NCORES) for c in range(NCORES + 1)]

    # segments by src bank only; dst goes through a two-stage gather
    core_data = []
    for c in range(NCORES):
        eidx = order[bounds[c]:bounds[c + 1]]
        seg = src[eidx] // bank
        segs = []
        for s in range(NBANKS):
            sel = eidx[seg == s]           # still rel-sorted (stable mask)
            r = rel[sel]
            n = len(sel)
            if n:
                change = np.flatnonzero(np.diff(r)) + 1
                starts = np.concatenate([[0], change])
                ends = np.concatenate([change, [n]])
                lens = ends - starts
                padlens = ((lens + P - 1) // P) * P
                offs = np.concatenate([[0], np.cumsum(padlens)])
                total = int(offs[-1])
                se = np.full(total, -1, np.int64)
                pos = (np.arange(n) - np.repeat(starts, lens)
                       + np.repeat(offs[:-1], lens))
                se[pos] = sel
                sr = np.repeat(r[starts], padlens)
            else:
                se = np.zeros(0, np.int64)
                sr = np.zeros(0, np.int64)
            segs.append((se, sr))
        core_data.append(segs)

    caps = []
    for s in range(NBANKS):
        cap = max(len(core_data[c][s][0]) for c in range(NCORES)) // P
        caps.append(int(cap))
    K = sum(caps)
    pad_tiles = (-K) % TILE_GROUP  # compute loop works in groups of 8 tiles
    caps[-1] += pad_tiles
    K += pad_tiles
    seg_tile0 = np.concatenate([[0], np.cumsum(caps)]).astype(int)

    nslot = K * P
    src_loc = np.zeros((NCORES, nslot), np.int64)   # bank-local src idx
    dst_all = np.zeros((NCORES, nslot), np.int64)   # global dst ids
    dst_valid = np.zeros((NCORES, nslot), bool)
    relc_all = np.zeros((NCORES, K), np.int64)      # relation per tile
    slot_edge_full = np.full((NCORES, nslot), -1, np.int64)

    for c in range(NCORES):
        for s in range(NBANKS):
            se, sr = core_data[c][s]
            a = seg_tile0[s] * P
            m = len(se)
            slot_edge_full[c, a:a + m] = se
            valid = se >= 0
            sl = np.zeros(m, np.int64)
            sl[valid] = src[se[valid]] - s * bank
            src_loc[c, a:a + m] = sl
            dl = np.zeros(m, np.int64)
            dl[valid] = dst[se[valid]]
            dst_all[c, a:a + m] = dl
            dst_valid[c, a:a + m] = valid
            nt = m // P
            if nt:
                relc_all[c, seg_tile0[s]:seg_tile0[s] + nt] = \
                    sr.reshape(nt, P)[:, 0]

    # dst stage A: per dst bank, the slots needing that bank, compacted.
    # Per-bank capacity maxed over cores; total rows CT (scratch size).
    bank_cnt = np.zeros((NCORES, NBANKS), np.int64)
    for c in range(NCORES):
        for b in range(NBANKS):
            sel = dst_valid[c] & (dst_all[c] // bank == b)
            bank_cnt[c, b] = len(np.unique(dst_all[c, sel]))
    bcaps = [int(-(-bank_cnt[:, b].max() // P) * P) for b in range(NBANKS)]
    brow0 = np.concatenate([[0], np.cumsum(bcaps)]).astype(int)
    CT = int(brow0[-1])  # scratch rows; must fit int16 for stage B
    assert CT <= 32767 and CT % P == 0

    dstA_loc = np.zeros((NCORES, CT), np.int64)   # stage-A bank-local ids
    dstB_row = np.zeros((NCORES, nslot), np.int64)  # stage-B scratch rows
    BC = CT // P  # scratch free blocks per partition
    for c in range(NCORES):
        for b in range(NBANKS):
            sel = np.flatnonzero(dst_valid[c] & (dst_all[c] // bank == b))
            # dedup: gather each distinct row once, point dup slots at it
            uniq, inv = np.unique(dst_all[c, sel], return_inverse=True)
            g = brow0[b] + np.arange(len(uniq))    # stage-A request index
            dstA_loc[c, g] = uniq - b * bank
            # request g lands at scratch flat row (g%128)*BC + g//128
            gslot = g[inv]
            dstB_row[c, sel] = (gslot % P) * BC + gslot // P

    def to_idx_tile(flat):
        n = flat.shape[1]
        blk = flat.reshape(NCORES, n // 16, 16).transpose(0, 2, 1)
        return np.tile(blk, (1, 8, 1)).astype(np.int16)

    src_idx = to_idx_tile(src_loc)
    dstA_idx = to_idx_tile(dstA_loc)
    dstB_idx = to_idx_tile(dstB_row)

    def chunk_plan(ranges, head=()):
        """head: sizes for the leading chunks of the first range (smaller
        chunks let the consumer start sooner)."""
        plan = []
        first = True
        for tag, a, b in ranges:
            g0 = a
            if first:
                for h in head:
                    if g0 + h > b:
                        break
                    plan.append((tag, g0, h))
                    g0 += h
                first = False
            while g0 < b:
                n = min(CHUNK, b - g0)
                plan.append((tag, g0, n))
                g0 += n
        return tuple(plan)

    # small leading src chunks: the PE pipeline (transpose->matmul) can
    # start as soon as the first 256 rows land instead of waiting ~9us
    src_plan = chunk_plan(
        [(s, seg_tile0[s] * P, seg_tile0[s + 1] * P) for s in range(NBANKS)],
        head=(256, 256, 512))
    dstA_plan = chunk_plan(
        [(b, brow0[b], brow0[b + 1]) for b in range(NBANKS)])
    # finer stage-B chunks: the per-supertile DVE stage consumes dst in
    # 1024-slot groups, so 512-row chunks smooth the tail
    dstB_plan = tuple(
        (0, g0, min(512, nslot - g0)) for g0 in range(0, nslot, 512))

    return (src_idx, dstA_idx, dstB_idx, relc_all, slot_edge_full, K, CT,
            src_plan, dstA_plan, dstB_plan, E)


def _build(K, CT, num_nodes, src_plan, dstA_plan, dstB_plan):
    nc = bacc.Bacc("TRN2", target_bir_lowering=False, debug=False,
                   num_devices=NCORES, num_swdge_queues=NQ)
    f32, i16 = mybir.dt.float32, mybir.dt.int16
    bank = -(-num_nodes // NBANKS)
    nslot = K * P
    BC = CT // P
    HG = TILE_GROUP // 2  # tiles per transpose/copy batch (one PSUM bank)

    node = nc.dram_tensor("node_emb", [num_nodes, DIM], f32,
                          kind="ExternalInput")
    wt_d = nc.dram_tensor("w_tile", [DIM, K * DIM], f32,
                          kind="ExternalInput")
    sidx_d = nc.dram_tensor("src_idx", [P, nslot // 16], i16,
                            kind="ExternalInput")
    daidx_d = nc.dram_tensor("dstA_idx", [P, CT // 16], i16,
                             kind="ExternalInput")
    dbidx_d = nc.dram_tensor("dstB_idx", [P, nslot // 16], i16,
                             kind="ExternalInput")
    out_d = nc.dram_tensor("scores", [P, K], f32, kind="ExternalOutput")

    with TileContext(nc) as tc:
        with (
            tc.tile_pool(name="persist", bufs=1) as persist,
            tc.tile_pool(name="tsb", bufs=6) as tsb_pool,
            tc.tile_pool(name="pbig", bufs=4) as pbig_pool,
            tc.tile_pool(name="dram", bufs=1, space="DRAM") as dram_pool,
            tc.tile_pool(name="tpsum", bufs=2, space="PSUM") as tpsum_pool,
            tc.tile_pool(name="upsum", bufs=6, space="PSUM") as upsum_pool,
        ):
            sidx = persist.tile([P, nslot // 16], i16, tag="sidx")
            daidx = persist.tile([P, CT // 16], i16, tag="daidx")
            dbidx = persist.tile([P, nslot // 16], i16, tag="dbidx")
            ident = persist.tile([P, P], f32, tag="ident")
            src_g = persist.tile([P, K * DIM], f32, tag="src_g")
            dstA = persist.tile([P, BC * DIM], f32, tag="dstA")
            dst_g = persist.tile([P, K * DIM], f32, tag="dst_g")
            w_g = persist.tile([DIM, K * DIM], f32, tag="w_g")
            scores = persist.tile([P, K], f32, tag="scores")
            scratch = dram_pool.tile([P, BC * DIM], f32, tag="scratch")

            nc.sync.dma_start(out=sidx[:], in_=sidx_d[:])
            nc.sync.dma_start(out=daidx[:], in_=daidx_d[:])
            nc.sync.dma_start(out=dbidx[:], in_=dbidx_d[:])
            nc.sync.dma_start(out=w_g[:], in_=wt_d[:])
            make_identity(nc, ident[:])

            # Interleave src and dst-stage-A chunks so the dst pipeline
            # drains early and stage B can start while src still gathers.
            # queue_num must follow the global SWDGE round-robin (Tile locks
            # DMA sem lanes to queues by instruction order).
            qn = 0
            src_items = [("s",) + it for it in src_plan]
            dstA_items = [("a",) + it for it in dstA_plan]
            inter = []
            na, nb = len(dstA_items), len(src_items)
            ia = ib = 0
            while ia < na or ib < nb:
                if ia < na:
                    inter.append(dstA_items[ia]); ia += 1
                if ib < nb:
                    inter.append(src_items[ib]); ib += 1
            for kind, bnk, g0, n in inter:
                hi = min(num_nodes, (bnk + 1) * bank)
                g_tile, idx_tile = ((src_g, sidx) if kind == "s"
                                    else (dstA, daidx))
                nc.gpsimd.dma_gather(
                    g_tile[:, (g0 // P) * DIM:((g0 + n) // P) * DIM]
                    .rearrange("p (t d) -> p t d", d=DIM),
                    node[bnk * bank:hi, :],
                    idx_tile[:, g0 // 16:(g0 + n) // 16],
                    n, n, DIM,
                    queue_num=qn % NQ,
                )
                qn += 1
            # stage A -> DRAM scratch (sequential, HWDGE path)
            nc.sync.dma_start(out=scratch[:], in_=dstA[:])
            # dst stage B: regather scratch rows into slot order
            scratch_rows = scratch[:].rearrange("a (b c) -> (a b) c", c=DIM)
            for _, g0, n in dstB_plan:
                nc.gpsimd.dma_gather(
                    dst_g[:, (g0 // P) * DIM:((g0 + n) // P) * DIM]
                    .rearrange("p (t d) -> p t d", d=DIM),
                    scratch_rows,
                    dbidx[:, g0 // 16:(g0 + n) // 16],
                    n, n, DIM,
                    queue_num=qn % NQ,
                )
                qn += 1

            for st in range(K // TILE_GROUP):
                t0 = st * TILE_GROUP
                # 4 pair-transposes ([128,128] -> [dA|dB, e]) into one PSUM
                # bank, then two strided ACT copies deinterleave the halves
                # into a base-partition-0 srcT buffer [64, 8*128].
                tp = tpsum_pool.tile([P, HG * P], f32, tag="tp")
                for q in range(HG):
                    c0 = (t0 + 2 * q) * DIM
                    nc.tensor.transpose(
                        out=tp[:, q * P:(q + 1) * P],
                        in_=src_g[:, c0:c0 + 2 * DIM],
                        identity=ident[:],
                    )
                tsb = tsb_pool.tile([DIM, TILE_GROUP * P], f32, tag="tsb")
                tsb_v = tsb[:].rearrange("p (t a c) -> p a t c", a=2, c=P)
                tp_v = tp[:].rearrange("p (q c) -> p q c", c=P)
                nc.scalar.copy(out=tsb_v[:, 0], in_=tp_v[0:DIM])
                nc.scalar.copy(out=tsb_v[:, 1], in_=tp_v[DIM:P])

                u = upsum_pool.tile([P, TILE_GROUP * DIM], f32, tag="u")
                for h in range(TILE_GROUP):
                    j = t0 + h
                    nc.tensor.matmul(
                        out=u[:, h * DIM:(h + 1) * DIM],
                        lhsT=tsb[:, h * P:(h + 1) * P],
                        rhs=w_g[:, j * DIM:(j + 1) * DIM],
                        start=True,
                        stop=True,
                    )
                pbig = pbig_pool.tile([P, TILE_GROUP * DIM], f32, tag="pbig")
                nc.vector.tensor_mul(
                    out=pbig[:],
                    in0=u[:],
                    in1=dst_g[:, t0 * DIM:(t0 + TILE_GROUP) * DIM],
                )
                nc.vector.reduce_sum(
                    out=scores[:, t0:t0 + TILE_GROUP],
                    in_=pbig[:].rearrange("p (t k) -> p t k", k=DIM),
                    axis=mybir.AxisListType.X,
                )

            nc.sync.dma_start(out=out_d[:], in_=scores[:])

    nc.compile()
    return nc


def kernel(triplets, node_emb, W):
    global LAST_RESULT
    node = np.ascontiguousarray(np.asarray(node_emb, dtype=np.float32))
    Wf = np.ascontiguousarray(np.asarray(W, dtype=np.float32))
    num_nodes = node.shape[0]

    (src_idx, dstA_idx, dstB_idx, relc_all, slot_edge, K, CT,
     src_plan, dstA_plan, dstB_plan, E) = _prepare(triplets, num_nodes)

    cache_key = (K, CT, num_nodes, src_plan, dstA_plan, dstB_plan)
    if cache_key not in _BUILD_CACHE:
        _BUILD_CACHE[cache_key] = _build(K, CT, num_nodes, src_plan,
                                         dstA_plan, dstB_plan)
    nc = _BUILD_CACHE[cache_key]

    in_maps = []
    for c in range(NCORES):
        # per-tile W: [K, 64, 64] -> [64, K*64] with w[d, j*64+k] = W[rel_j,d,k]
        wt = np.ascontiguousarray(
            Wf[relc_all[c]].transpose(1, 0, 2).reshape(DIM, K * DIM))
        in_maps.append({
            "node_emb": node,
            "w_tile": wt,
            "src_idx": np.ascontiguousarray(src_idx[c]),
            "dstA_idx": np.ascontiguousarray(dstA_idx[c]),
            "dstB_idx": np.ascontiguousarray(dstB_idx[c]),
        })

    res = run_bass_kernel_spmd(nc, in_maps, list(range(NCORES)), trace=TRACE)
    LAST_RESULT = res

    out = np.zeros(E, np.float32)
    for c in range(NCORES):
        sc = np.asarray(res.results[c]["scores"])  # [P, K]
        flat = sc.T.ravel()                        # index j*P+p = slot s
        se = slot_edge[c]
        valid = se >= 0
        out[se[valid]] = flat[valid]
    return out


# revision 31
# speedup vs baseline: 1.3556x; 1.0303x over previous
"""DistMult edge scoring on Trainium2 (8 NeuronCores).

score_e = src_emb[e]^T @ W[rel_e] @ dst_emb[e]   for E=100k edges.

Strategy
--------
Host (index-space preprocessing only — no embedding data is gathered on
host):
  - Sort edges by relation, shard the sorted list contiguously across the
    8 cores (data-parallel over edges).
  - Per core, bucket edges into 16 segments by (src_bank, dst_bank) where
    a bank is a num_nodes/4-row range of the node table (node ids must fit
    the int16 index of the HW gather instruction). Within a segment, edges
    stay relation-sorted and each relation run is padded to a multiple of
    128 so every 128-edge tile is single-relation and single-bank on both
    endpoints. Segment capacities are maxed across cores so all cores run
    one SPMD program.
  - The small relation matrices (64x64x64 = 1MB) are expanded per tile on
    host and streamed over the HWDGE path, keeping the SWDGE gather
    queues free for the 25.6MB node table.

Device (per core, SPMD):
  - dma_gather (int16, 4 SWDGE queues, <=1024 rows/instruction) of
    src/dst embedding rows, per bank segment.
  - per tile: PE transpose src -> srcT [d, e] (base partition 0);
    PE matmul U[e,k] = sum_d srcT[d,e] * W[r,d,k];
    per 8 tiles: DVE mul by dst and reduce over k -> scores.
Host: drop pad slots, unsort scores to the original edge order.
"""

import numpy as np

import concourse.bacc as bacc
import concourse.mybir as mybir
from concourse.bass_utils import run_bass_kernel_spmd
from concourse.masks import make_identity
from concourse.tile import TileContext

NCORES = 8
P = 128          # SBUF partitions / edges per tile
DIM = 64         # embedding dim
NUM_RELS = 64
NBANKS = 4
TILE_GROUP = 8   # tiles per compute super-tile (one PSUM bank: 8*64 f32)
CHUNK = 1024     # max indices per dma_gather instruction (SWDGE ring limit)
NQ = 4           # SWDGE queues

TRACE = False
LAST_RESULT = None

_BUILD_CACHE = {}


def _prepare(triplets, num_nodes):
    """Index-space prep. Returns per-core int16 gather streams + unsort map.

    Slot s (= tile j * 128 + p) of core c holds the edge at padded position
    s; tiles are relation- and bank-pure by construction.
    """
    t = np.asarray(triplets)
    E = t.shape[0]
    src = t[:, 0].astype(np.int64)
    rel = t[:, 1].astype(np.int64)
    dst = t[:, 2].astype(np.int64)
    bank = -(-num_nodes // NBANKS)  # equal banks, < 32768 for int16
    assert bank <= 32767

    order = np.argsort(rel, kind="stable")
    bounds = [round(c * E / NCORES) for c in range(NCORES + 1)]

    # segments by src bank only; dst goes through a two-stage gather
    core_data = []
    for c in range(NCORES):
        eidx = order[bounds[c]:bounds[c + 1]]
        seg = src[eidx] // bank
        segs = []
        for s in range(NBANKS):
            sel = eidx[seg == s]           # still rel-sorted (stable mask)
            r = rel[sel]
            n = len(sel)
            if n:
                change = np.flatnonzero(np.diff(r)) + 1
                starts = np.concatenate([[0], change])
                ends = np.concatenate([change, [n]])
                lens = ends - starts
                padlens = ((lens + P - 1) // P) * P
                offs = np.concatenate([[0], np.cumsum(padlens)])
                total = int(offs[-1])
                se = np.full(total, -1, np.int64)
                pos = (np.arange(n) - np.repeat(starts, lens)
                       + np.repeat(offs[:-1], lens))
                se[pos] = sel
                sr = np.repeat(r[starts], padlens)
            else:
                se = np.zeros(0, np.int64)
                sr = np.zeros(0, np.int64)
            segs.append((se, sr))
        core_data.append(segs)

    caps = []
    for s in range(NBANKS):
        cap = max(len(core_data[c][s][0]) for c in range(NCORES)) // P
        caps.append(int(cap))
    K = sum(caps)
    pad_tiles = (-K) % TILE_GROUP  # compute loop works in groups of 8 tiles
    caps[-1] += pad_tiles
    K += pad_tiles
    seg_tile0 = np.concatenate([[0], np.cumsum(caps)]).astype(int)

    nslot = K * P
    src_loc = np.zeros((NCORES, nslot), np.int64)   # bank-local src idx
    dst_all = np.zeros((NCORES, nslot), np.int64)   # global dst ids
    dst_valid = np.zeros((NCORES, nslot), bool)
    relc_all = np.zeros((NCORES, K), np.int64)      # relation per tile
    slot_edge_full = np.full((NCORES, nslot), -1, np.int64)

    for c in range(NCORES):
        for s in range(NBANKS):
            se, sr = core_data[c][s]
            a = seg_tile0[s] * P
            m = len(se)
            slot_edge_full[c, a:a + m] = se
            valid = se >= 0
            sl = np.zeros(m, np.int64)
            sl[valid] = src[se[valid]] - s * bank
            src_loc[c, a:a + m] = sl
            dl = np.zeros(m, np.int64)
            dl[valid] = dst[se[valid]]
            dst_all[c, a:a + m] = dl
            dst_valid[c, a:a + m] = valid
            nt = m // P
            if nt:
                relc_all[c, seg_tile0[s]:seg_tile0[s] + nt] = \
                    sr.reshape(nt, P)[:, 0]

    # dst stage A: per dst bank, the slots needing that bank, compacted.
    # Per-bank capacity maxed over cores; total rows CT (scratch size).
    bank_cnt = np.zeros((NCORES, NBANKS), np.int64)
    for c in range(NCORES):
        for b in range(NBANKS):
            sel = dst_valid[c] & (dst_all[c] // bank == b)
            bank_cnt[c, b] = len(np.unique(dst_all[c, sel]))
    bcaps = [int(-(-bank_cnt[:, b].max() // P) * P) for b in range(NBANKS)]
    brow0 = np.concatenate([[0], np.cumsum(bcaps)]).astype(int)
    CT = int(brow0[-1])  # scratch rows; must fit int16 for stage B
    assert CT <= 32767 and CT % P == 0

    dstA_loc = np.zeros((NCORES, CT), np.int64)   # stage-A bank-local ids
    dstB_row = np.zeros((NCORES, nslot), np.int64)  # stage-B scratch rows
    BC = CT // P  # scratch free blocks per partition
    for c in range(NCORES):
        for b in range(NBANKS):
            sel = np.flatnonzero(dst_valid[c] & (dst_all[c] // bank == b))
            # dedup: gather each distinct row once, point dup slots at it
            uniq, inv = np.unique(dst_all[c, sel], return_inverse=True)
            g = brow0[b] + np.arange(len(uniq))    # stage-A request index
            dstA_loc[c, g] = uniq - b * bank
            # request g lands at scratch flat row (g%128)*BC + g//128
            gslot = g[inv]
            dstB_row[c, sel] = (gslot % P) * BC + gslot // P

    def to_idx_tile(flat):
        n = flat.shape[1]
        blk = flat.reshape(NCORES, n // 16, 16).transpose(0, 2, 1)
        return np.tile(blk, (1, 8, 1)).astype(np.int16)

    src_idx = to_idx_tile(src_loc)
    dstA_idx = to_idx_tile(dstA_loc)
    dstB_idx = to_idx_tile(dstB_row)

    def chunk_plan(ranges, head=()):
        """head: sizes for the leading chunks of the first range (smaller
        chunks let the consumer start sooner)."""
        plan = []
        first = True
        for tag, a, b in ranges:
            g0 = a
            if first:
                for h in head:
                    if g0 + h > b:
                        break
                    plan.append((tag, g0, h))
                    g0 += h
                first = False
            while g0 < b:
                n = min(CHUNK, b - g0)
                plan.append((tag, g0, n))
                g0 += n
        return tuple(plan)

    # small leading src chunks: the PE pipeline (transpose->matmul) can
    # start as soon as the first 256 rows land instead of waiting ~9us
    src_plan = chunk_plan(
        [(s, seg_tile0[s] * P, seg_tile0[s + 1] * P) for s in range(NBANKS)],
        head=(256, 256, 512))
    dstA_plan = chunk_plan(
        [(b, brow0[b], brow0[b + 1]) for b in range(NBANKS)])
    # finer stage-B chunks: the per-supertile DVE stage consumes dst in
    # 1024-slot groups, so 512-row chunks smooth the tail
    dstB_plan = tuple(
        (0, g0, min(512, nslot - g0)) for g0 in range(0, nslot, 512))

    return (src_idx, dstA_idx, dstB_idx, relc_all, slot_edge_full, K, CT,
            src_plan, dstA_plan, dstB_plan, E)


def _build(K, CT, num_nodes, src_plan, dstA_plan, dstB_plan):
    nc = bacc.Bacc("TRN2", target_bir_lowering=False, debug=False,
                   num_devices=NCORES, num_swdge_queues=NQ)
    f32, i16 = mybir.dt.float32, mybir.dt.int16
    bank = -(-num_nodes // NBANKS)
    nslot = K * P
    BC = CT // P
    HG = TILE_GROUP // 2  # tiles per transpose/copy batch (one PSUM bank)

    node = nc.dram_tensor("node_emb", [num_nodes, DIM], f32,
                          kind="ExternalInput")
    wt_d = nc.dram_tensor("w_tile", [DIM, K * DIM], f32,
                          kind="ExternalInput")
    sidx_d = nc.dram_tensor("src_idx", [P, nslot // 16], i16,
                            kind="ExternalInput")
    daidx_d = nc.dram_tensor("dstA_idx", [P, CT // 16], i16,
                             kind="ExternalInput")
    dbidx_d = nc.dram_tensor("dstB_idx", [P, nslot // 16], i16,
                             kind="ExternalInput")
    out_d = nc.dram_tensor("scores", [P, K], f32, kind="ExternalOutput")

    with TileContext(nc) as tc:
        with (
            tc.tile_pool(name="persist", bufs=1) as persist,
            tc.tile_pool(name="tsb", bufs=6) as tsb_pool,
            tc.tile_pool(name="pbig", bufs=4) as pbig_pool,
            tc.tile_pool(name="dram", bufs=1, space="DRAM") as dram_pool,
            tc.tile_pool(name="tpsum", bufs=2, space="PSUM") as tpsum_pool,
            tc.tile_pool(name="upsum", bufs=6, space="PSUM") as upsum_pool,
        ):
            sidx = persist.tile([P, nslot // 16], i16, tag="sidx")
            daidx = persist.tile([P, CT // 16], i16, tag="daidx")
            dbidx = persist.tile([P, nslot // 16], i16, tag="dbidx")
            ident = persist.tile([P, P], f32, tag="ident")
            src_g = persist.tile([P, K * DIM], f32, tag="src_g")
            dstA = persist.tile([P, BC * DIM], f32, tag="dstA")
            dst_g = persist.tile([P, K * DIM], f32, tag="dst_g")
            w_g = persist.tile([DIM, K * DIM], f32, tag="w_g")
            scores = persist.tile([P, K], f32, tag="scores")
            scratch = dram_pool.tile([P, BC * DIM], f32, tag="scratch")

            nc.sync.dma_start(out=sidx[:], in_=sidx_d[:])
            nc.sync.dma_start(out=daidx[:], in_=daidx_d[:])
            nc.sync.dma_start(out=dbidx[:], in_=dbidx_d[:])
            nc.sync.dma_start(out=w_g[:], in_=wt_d[:])
            make_identity(nc, ident[:])

    # Issue order: small src head chunks first (unblock the PE pipeline),
            # then ALL dst-stage-A chunks (so the scratch hop + stage B start
            # early and overlap the src back-half), then the remaining src.
            # queue_num must follow the global SWDGE round-robin (Tile locks
            # DMA sem lanes to queues by instruction order).
            qn = 0
            src_items = [("s",) + it for it in src_plan]
            dstA_items = [("a",) + it for it in dstA_plan]
            nhead = sum(1 for _, _, n in src_plan if n < CHUNK and n <= 512)
            inter = (src_items[:nhead] + dstA_items + src_items[nhead:])
            for kind, bnk, g0, n in inter:
                hi = min(num_nodes, (bnk + 1) * bank)
                g_tile, idx_tile = ((src_g, sidx) if kind == "s"
                                    else (dstA, daidx))
                nc.gpsimd.dma_gather(
                    g_tile[:, (g0 // P) * DIM:((g0 + n) // P) * DIM]
                    .rearrange("p (t d) -> p t d", d=DIM),
                    node[bnk * bank:hi, :],
                    idx_tile[:, g0 // 16:(g0 + n) // 16],
                    n, n, DIM,
                    queue_num=qn % NQ,
                )
                qn += 1
            # stage A -> DRAM scratch (sequential, HWDGE path)
            nc.sync.dma_start(out=scratch[:], in_=dstA[:])
            # dst stage B: regather scratch rows into slot order
            scratch_rows = scratch[:].rearrange("a (b c) -> (a b) c", c=DIM)
            for _, g0, n in dstB_plan:
                nc.gpsimd.dma_gather(
                    dst_g[:, (g0 // P) * DIM:((g0 + n) // P) * DIM]
                    .rearrange("p (t d) -> p t d", d=DIM),
                    scratch_rows,
                    dbidx[:, g0 // 16:(g0 + n) // 16],
                    n, n, DIM,
                    queue_num=qn % NQ,
                )
                qn += 1

            for st in range(K // TILE_GROUP):
                t0 = st * TILE_GROUP
                # 4 pair-transposes ([128,128] -> [dA|dB, e]) into one PSUM
                # bank, then two strided ACT copies deinterleave the halves
                # into a base-partition-0 srcT buffer [64, 8*128].
                tp = tpsum_pool.tile([P, HG * P], f32, tag="tp")
                for q in range(HG):
                    c0 = (t0 + 2 * q) * DIM
                    nc.tensor.transpose(
                        out=tp[:, q * P:(q + 1) * P],
                        in_=src_g[:, c0:c0 + 2 * DIM],
                        identity=ident[:],
                    )
                tsb = tsb_pool.tile([DIM, TILE_GROUP * P], f32, tag="tsb")
                tsb_v = tsb[:].rearrange("p (t a c) -> p a t c", a=2, c=P)
                tp_v = tp[:].rearrange("p (q c) -> p q c", c=P)
                nc.scalar.copy(out=tsb_v[:, 0], in_=tp_v[0:DIM])
                nc.scalar.copy(out=tsb_v[:, 1], in_=tp_v[DIM:P])

                u = upsum_pool.tile([P, TILE_GROUP * DIM], f32, tag="u")
                for h in range(TILE_GROUP):
                    j = t0 + h
                    nc.tensor.matmul(
                        out=u[:, h * DIM:(h + 1) * DIM],
                        lhsT=tsb[:, h * P:(h + 1) * P],
                        rhs=w_g[:, j * DIM:(j + 1) * DIM],
                        start=True,
                        stop=True,
                    )
                pbig = pbig_pool.tile([P, TILE_GROUP * DIM], f32, tag="pbig")
                nc.vector.tensor_mul(
                    out=pbig[:],
                    in0=u[:],
                    in1=dst_g[:, t0 * DIM:(t0 + TILE_GROUP) * DIM],
                )
                nc.vector.reduce_sum(
                    out=scores[:, t0:t0 + TILE_GROUP],
                    in_=pbig[:].rearrange("p (t k) -> p t k", k=DIM),
                    axis=mybir.AxisListType.X,
                )

            nc.sync.dma_start(out=out_d[:], in_=scores[:])

    nc.compile()
    return nc


def kernel(triplets, node_emb, W):
    global LAST_RESULT
    node = np.ascontiguousarray(np.asarray(node_emb, dtype=np.float32))
    Wf = np.ascontiguousarray(np.asarray(W, dtype=np.float32))
    num_nodes = node.shape[0]

    (src_idx, dstA_idx, dstB_idx, relc_all, slot_edge, K, CT,
     src_plan, dstA_plan, dstB_plan, E) = _prepare(triplets, num_nodes)

    cache_key = (K, CT, num_nodes, src_plan, dstA_plan, dstB_plan)
    if cache_key not in _BUILD_CACHE:
        _BUILD_CACHE[cache_key] = _build(K, CT, num_nodes, src_plan,
                                         dstA_plan, dstB_plan)
    nc = _BUILD_CACHE[cache_key]

    in_maps = []
    for c in range(NCORES):
        # per-tile W: [K, 64, 64] -> [64, K*64] with w[d, j*64+k] = W[rel_j,d,k]
        wt = np.ascontiguousarray(
            Wf[relc_all[c]].transpose(1, 0, 2).reshape(DIM, K * DIM))
        in_maps.append({
            "node_emb": node,
            "w_tile": wt,
            "src_idx": np.ascontiguousarray(src_idx[c]),
            "dstA_idx": np.ascontiguousarray(dstA_idx[c]),
            "dstB_idx": np.ascontiguousarray(dstB_idx[c]),
        })

    res = run_bass_kernel_spmd(nc, in_maps, list(range(NCORES)), trace=TRACE)
    LAST_RESULT = res

    out = np.zeros(E, np.float32)
    for c in range(NCORES):
        sc = np.asarray(res.results[c]["scores"])  # [P, K]
        flat = sc.T.ravel()                        # index j*P+p = slot s
        se = slot_edge[c]
        valid = se >= 0
        out[se[valid]] = flat[valid]
    return out
